# revision 1
# baseline (speedup 1.0000x reference)
"""AttentiveTransformer (Dense + BN(inference) + prior-scale + sparsemax) on 8 trn2 cores.

Math (per reference):
    z   = (x @ W + b) * inv + (beta - mm*inv),  inv = gamma/sqrt(mv+eps)
    z   = z * prior_scales
    out = sparsemax(z)  (rowwise simplex projection)

Strategy:
  - Host folds BN scale into W (W' = W*inv) and all additive terms into a
    per-feature bias b' = beta + (b - mm)*inv.   z = x@W' + b' (* prior).
  - Data-parallel over batch: 8192 rows -> 8 cores x 1024 rows.
  - Per core: z computed via fp32r matmuls (x.T tiles stationary, W' moving),
    bias added during the PSUM->SBUF drain (fused into tensor_tensor_reduce,
    which also produces per-chunk row maxes).
  - sparsemax tau found by Michelot fixed-point iteration started at
    tau0 = rowmax - 1 (a provable lower bound of tau*):
        tau' = tau + (sum(relu(z - tau)) - 1) / count(z > tau)
    f-pass on ScalarE (activation Relu with per-partition bias=-tau and
    accumulate), count-pass on VectorE (tensor_scalar is_gt with add-reduce
    accumulator).  Iteration count is fixed/unrolled; converged rows are
    idempotent.  Host verifies row sums afterwards and falls back to a
    higher iteration count in the (never observed) case of non-convergence.
"""

import sys

for _p in ("/opt/trn_rl_repo",):
    if _p not in sys.path:
        sys.path.append(_p)

from contextlib import ExitStack

import numpy as np

import concourse.bacc as bacc
import concourse.bass as bass
import concourse.mybir as mybir
import concourse.tile as tile
from concourse import bass_utils

B, F = 8192, 2048
NCORES = 8
BS = B // NCORES          # rows per core
P = 128                   # SBUF partitions
KC = F // P               # contraction chunks
NFREE = 512               # matmul moving free dim (one PSUM bank)
NCHUNK = F // NFREE       # output feature chunks
BN_EPS = 1e-3
FMIN = -3.0e38
DEFAULT_NITERS = 4

f32 = mybir.dt.float32
f32r = mybir.dt.float32r
bf16 = mybir.dt.bfloat16


def build_program(with_prior: bool, niters: int, bs: int = BS, f: int = F):
    """Build the per-core Bass program (SPMD: every core runs this)."""
    kc = f // P
    nchunk = max(1, f // NFREE)
    nfree = f // nchunk
    mt = bs // P

    nc = bacc.Bacc()
    xt = nc.dram_tensor("xt", [f, bs], f32, kind="ExternalInput")
    wp = nc.dram_tensor("wp", [f, f], f32, kind="ExternalInput")
    bprep = nc.dram_tensor("bprep", [P, f], f32, kind="ExternalInput")
    prior = None
    if with_prior:
        prior = nc.dram_tensor("prior", [bs, f], f32, kind="ExternalInput")
    out = nc.dram_tensor("out", [bs, f], f32, kind="ExternalOutput")

    relu = mybir.ActivationFunctionType.Relu
    AO = mybir.AluOpType

    with tile.TileContext(nc) as tc, ExitStack() as ctx:
        consts = ctx.enter_context(tc.tile_pool(name="consts", bufs=1))
        wpool = ctx.enter_context(tc.tile_pool(name="w", bufs=1))
        xpool = ctx.enter_context(tc.tile_pool(name="x", bufs=2))
        zpool = ctx.enter_context(tc.tile_pool(name="z", bufs=3))
        spool = ctx.enter_context(tc.tile_pool(name="scr", bufs=2))
        vpool = ctx.enter_context(tc.tile_pool(name="vec", bufs=8))
        psum = ctx.enter_context(tc.tile_pool(name="psum", bufs=6, space="PSUM"))
        fpool = ctx.enter_context(tc.tile_pool(name="fscr", bufs=1))
        kpool = ctx.enter_context(tc.tile_pool(name="kscr", bufs=1))
        prpool = None
        if with_prior:
            prpool = ctx.enter_context(tc.tile_pool(name="pr", bufs=2))

        # bias, replicated across partitions on host: [128, f]
        bp_t = consts.tile([P, f], f32)
        nc.sync.dma_start(out=bp_t, in_=bprep[:, :])

        ones_t = bprow = None
        if with_prior:
            ones_t = consts.tile([1, P], f32r)
            nc.vector.memset(ones_t, 1.0)
            bprow = consts.tile([1, f], f32r)
            nc.sync.dma_start(out=bprow, in_=bprep[0:1, :].bitcast(f32r))

        # x.T viewed as [p, kchunk, batch] for one-DMA tile loads
        xt_r = xt.rearrange("(c p) b -> p c b", p=P)

        # first x tile loads before the bulk of W so compute starts early
        x0_t = xpool.tile([P, kc, P], f32r, tag="xt")
        nc.sync.dma_start(out=x0_t, in_=xt_r[:, :, 0:P].bitcast(f32r))

        # W' resident in SBUF as kc x nchunk tiles of [128, nfree], loaded
        # column-chunk-major so the first output chunk's weights arrive first
        w_t = [[None] * nchunk for _ in range(kc)]
        for c in range(nchunk):
            for k in range(kc):
                cs = slice(c * nfree, (c + 1) * nfree)
                wt = wpool.tile([P, nfree], f32r, tag=f"w{k}_{c}")
                nc.sync.dma_start(out=wt, in_=wp[k * P:(k + 1) * P, cs].bitcast(f32r))
                w_t[k][c] = wt

        for m in range(mt):
            if m == 0:
                x_t = x0_t
            else:
                x_t = xpool.tile([P, kc, P], f32r, tag="xt")
                nc.sync.dma_start(out=x_t, in_=xt_r[:, :, m * P:(m + 1) * P].bitcast(f32r))

            pr_t = None
            if with_prior:
                pr_t = prpool.tile([P, f], f32, tag="pr")
                nc.sync.dma_start(out=pr_t, in_=prior[m * P:(m + 1) * P, :])

            z_t = zpool.tile([P, f], f32, tag="z")
            for c in range(nchunk):
                ps = psum.tile([P, nfree], f32, tag="ps")
                cs = slice(c * nfree, (c + 1) * nfree)
                for k in range(kc):
                    nc.tensor.matmul(
                        ps,
                        x_t[:, k, :],
                        w_t[k][c],
                        start=(k == 0),
                        stop=(k == kc - 1 and not with_prior),
                    )
                if with_prior:
                    # bias via rank-1 matmul; drain applies prior multiply
                    nc.tensor.matmul(
                        ps,
                        ones_t,
                        bprow[:, cs],
                        start=False,
                        stop=True,
                    )
                    nc.vector.tensor_tensor(z_t[:, cs], ps, pr_t[:, cs], op=AO.mult)
                else:
                    # z = psum + bias
                    nc.vector.tensor_tensor(z_t[:, cs], ps, bp_t[:, cs], op=AO.add)

            # per-chunk maxes via tensor_scalar max-accumulate
            # (tensor_tensor_reduce faults the DVE on this walrus/ucode)
            mx = vpool.tile([P, nchunk], f32, tag="mx")
            for c in range(nchunk):
                cs = slice(c * nfree, (c + 1) * nfree)
                scr_m = spool.tile([P, nfree], bf16, tag="scrm")
                nc.vector.tensor_scalar(scr_m, z_t[:, cs], 0.0, None, op0=AO.add,
                                        op1=AO.max, accum_out=mx[:, c:c + 1])
            # top-2 of the 4 chunk maxes -> init bound
            # tau0 = max(m - 1, (m + s - 1)/2), both provable lower bounds of tau*
            pq = vpool.tile([P, 2], f32, tag="pq")   # p=max(a,b), q=min(a,b)
            rt = vpool.tile([P, 2], f32, tag="rt")
            nc.vector.tensor_tensor(pq[:, 0:1], mx[:, 0:1], mx[:, 1:2], op=AO.max)
            nc.vector.tensor_tensor(pq[:, 1:2], mx[:, 0:1], mx[:, 1:2], op=AO.min)
            nc.vector.tensor_tensor(rt[:, 0:1], mx[:, 2:3], mx[:, 3:4], op=AO.max)
            nc.vector.tensor_tensor(rt[:, 1:2], mx[:, 2:3], mx[:, 3:4], op=AO.min)
            mrow = vpool.tile([P, 1], f32, tag="mrow")
            nc.vector.tensor_tensor(mrow, pq[:, 0:1], rt[:, 0:1], op=AO.max)
            s2a = vpool.tile([P, 1], f32, tag="s2a")  # min(p, r)
            nc.vector.tensor_tensor(s2a, pq[:, 0:1], rt[:, 0:1], op=AO.min)
            s2b = vpool.tile([P, 1], f32, tag="s2b")  # max(q, t)
            nc.vector.tensor_tensor(s2b, pq[:, 1:2], rt[:, 1:2], op=AO.max)
            s2 = vpool.tile([P, 1], f32, tag="s2")    # second-largest
            nc.vector.tensor_tensor(s2, s2a, s2b, op=AO.max)
            b2 = vpool.tile([P, 1], f32, tag="b2")    # (m + s - 1) / 2
            nc.vector.tensor_tensor(b2, mrow, s2, op=AO.add)
            nc.vector.tensor_scalar(b2, b2, -1.0, 0.5, op0=AO.add, op1=AO.mult)
            b1 = vpool.tile([P, 1], f32, tag="b1")    # m - 1
            nc.vector.tensor_scalar(b1, mrow, -1.0, None, op0=AO.add)
            tau = vpool.tile([P, 1], f32, tag="tau")
            nc.vector.tensor_tensor(tau, b1, b2, op=AO.max)
            nt = vpool.tile([P, 1], f32, tag="nt")
            nc.vector.tensor_scalar(nt, tau, -1.0, None, op0=AO.mult)

            for _ in range(niters):
                # f = sum(relu(z - tau))   (ScalarE, accumulate)
                scr_f = fpool.tile([P, f], f32, tag="scrf")
                facc = vpool.tile([P, 1], f32, tag="facc")
                nc.scalar.activation(scr_f, z_t, relu, bias=nt, scale=1.0,
                                     accum_out=facc)
                # k = count(z > tau)   (VectorE, is_gt with add-reduce accum)
                scr_k = kpool.tile([P, f], bf16, tag="scrk")
                kacc = vpool.tile([P, 1], f32, tag="kacc")
                nc.vector.tensor_scalar(scr_k, z_t, tau, None,
                                        op0=AO.is_gt, op1=AO.add,
                                        accum_out=kacc)
                # tau' = tau + (f-1)/k
                rk = vpool.tile([P, 1], f32, tag="rk")
                nc.vector.reciprocal(rk, kacc)
                delta = vpool.tile([P, 1], f32, tag="delta")
                nc.vector.scalar_tensor_tensor(delta, facc, -1.0, rk,
                                               op0=AO.add, op1=AO.mult)
                tau2 = vpool.tile([P, 1], f32, tag="tau")
                nc.vector.tensor_tensor(tau2, tau, delta, op=AO.add)
                nt2 = vpool.tile([P, 1], f32, tag="nt")
                nc.vector.tensor_tensor(nt2, nt, delta, op=AO.subtract)
                tau, nt = tau2, nt2

            # final: out = relu(z - tau), in place, then store
            nc.scalar.activation(z_t, z_t, relu, bias=nt, scale=1.0)
            nc.sync.dma_start(out=out[m * P:(m + 1) * P, :], in_=z_t)

    nc.compile()
    return nc


_PROGRAMS: dict = {}


def _get_program(with_prior: bool, niters: int):
    key = (with_prior, niters)
    if key not in _PROGRAMS:
        _PROGRAMS[key] = build_program(with_prior, niters)
    return _PROGRAMS[key]


def _fold_host(W, b, gamma, beta, moving_mean, moving_var):
    inv = (gamma / np.sqrt(moving_var + np.float32(BN_EPS))).astype(np.float32)
    Wp = (W * inv[None, :]).astype(np.float32)
    bp = (beta + (b - moving_mean) * inv).astype(np.float32)
    return Wp, bp


def _run(with_prior: bool, niters: int, xT, Wp, bp_rep, prior):
    nc = _get_program(with_prior, niters)
    in_maps = []
    for c in range(NCORES):
        m = {
            "xt": np.ascontiguousarray(xT[:, c * BS:(c + 1) * BS]),
            "wp": Wp,
            "bprep": bp_rep,
        }
        if with_prior:
            m["prior"] = np.ascontiguousarray(prior[c * BS:(c + 1) * BS, :])
        in_maps.append(m)
    res = bass_utils.run_bass_kernel_spmd(nc, in_maps, core_ids=list(range(NCORES)))
    return np.concatenate([r["out"] for r in res.results], axis=0)


def kernel(inputs, W, b, gamma, beta, moving_mean, moving_var, prior_scales):
    inputs = np.ascontiguousarray(np.asarray(inputs, dtype=np.float32))
    W = np.ascontiguousarray(np.asarray(W, dtype=np.float32))
    b = np.asarray(b, dtype=np.float32)
    gamma = np.asarray(gamma, dtype=np.float32)
    beta = np.asarray(beta, dtype=np.float32)
    moving_mean = np.asarray(moving_mean, dtype=np.float32)
    moving_var = np.asarray(moving_var, dtype=np.float32)
    prior_scales = np.asarray(prior_scales, dtype=np.float32)

    Wp, bp = _fold_host(W, b, gamma, beta, moving_mean, moving_var)
    bp_rep = np.ascontiguousarray(np.broadcast_to(bp[None, :], (P, F)))
    xT = np.ascontiguousarray(inputs.T)

    # prior==1 exactly -> multiplying by it is an algebraic no-op; skip it.
    with_prior = not bool(np.all(prior_scales == np.float32(1.0)))

    out = _run(with_prior, DEFAULT_NITERS, xT, Wp, bp_rep, prior_scales)

    # sparsemax rows must sum to 1; if any row hasn't converged (never
    # observed for this data), redo with a conservative iteration count.
    rs = out.sum(axis=1, dtype=np.float64)
    if not np.all(np.abs(rs - 1.0) < 1e-3):
        out = _run(with_prior, 16, xT, Wp, bp_rep, prior_scales)
    return out

